# revision 28
# baseline (speedup 1.0000x reference)
"""GCN layer (message passing) on 8 Trainium2 NeuronCores via Bass/Tile.

out[b, n, :] = concat([W_lin @ (A @ x)[b, n] + b_lin, W_eye @ x[b, n] + b_eye])
with A the sparse adjacency given by (rows, cols, vals), x: [B, N, CIN].

Strategy (1D node partitioning, replicated graph features):
  - x is repacked host-side to xg[N, B*CIN] so one gathered row carries all
    batches for a node (1KB rows -> efficient DMA descriptors).
  - Nodes are grouped in chunks of 128 rows; chunks are assigned to
    (core, position) slots sorted by edge count so the SPMD-uniform schedule
    (max across cores per position) is tight. Per chunk, edges split into two
    col-range streams (dma_gather indices are int16 -> each stream addresses
    a <=25000-row view of xg).
  - Device, per chunk: bulk dma_gathers (<=1024 indices each -- larger calls
    crash the HW) fetch neighbor rows in wrap order [128/partition-tile];
    a one-hot selection sel[e, r] = (rowl[e] == r) * val[e] is built for all
    tiles in two batched DVE ops, and PE accumulates sel.T @ msg into PSUM,
    yielding (A@x)[128 rows, B*C] in fp32. PE transposes put channels
    on partitions; two block-diagonal [128, 64] matmuls per half apply
    W_lin/W_eye (all matmuls full-128 base-0: mixed lhsT base partitions
    crash the HW); bias add; contiguous store.
  - Host inverts the chunk permutation and reshapes to [B, N, 64].
    Measured: ~1.05 ms on 8 cores, bound by the Q7 SWDGE descriptor
    generation (~8.7 ns/gathered row).
"""

import numpy as np

import concourse.bacc as bacc
import concourse.mybir as mybir
import concourse.tile as tile
from concourse.bass_utils import run_bass_kernel_spmd

# Problem constants (hardcoded per contract)
B, N, CIN, HALF = 4, 50000, 64, 32
P = 128
NCORES = 8
D = B * CIN            # 256 (gathered row width)
DO = B * 2 * HALF      # 256 (output row width)
CSPLIT = 32768         # col split so gather indices fit in int16

F32 = mybir.dt.float32
BF16 = mybir.dt.bfloat16
I16 = mybir.dt.int16


def _schedule(rows, cols, vals, n, ncores):
    """Build the SPMD-uniform per-core gather/selection tables."""
    rows = np.asarray(rows, dtype=np.int64)
    cols = np.asarray(cols, dtype=np.int64)
    vals = np.asarray(vals, dtype=np.float32)

    nch_real = -(-n // P)
    nchg = -(-nch_real // ncores) * ncores      # pad #chunks to multiple of cores
    cpc = nchg // ncores                        # chunks per core
    n_pad = nchg * P

    g = rows // P                 # chunk of each edge
    q = rows % P                  # local row within chunk
    hi = (cols >= CSPLIT).astype(np.int64)

    # edges per (chunk, stream); assign chunks to (core, pos) sorted by edge
    # count so the per-position max across cores (the SPMD schedule) is tight.
    grp = g * 2 + hi
    cnt = np.bincount(grp, minlength=nchg * 2)
    cnt2 = cnt.reshape(nchg, 2)
    order = np.argsort(-cnt2.sum(1), kind="stable")
    asg = order.reshape(cpc, ncores)            # asg[pos, k] = global chunk
    pos_of = np.empty(nchg, np.int64)
    core_of = np.empty(nchg, np.int64)
    pos_of[order] = np.arange(nchg) // ncores
    core_of[order] = np.arange(nchg) % ncores

    # scheduled per-call index count: max across cores, 16-aligned
    ns2 = np.maximum(cnt2[asg].max(axis=1), 1)  # [cpc, 2]
    ns2 = (ns2 + 15) // 16 * 16
    TL = -(-ns2[:, 0] // P)                     # tiles actually touched
    TH = -(-ns2[:, 1] // P)
    T = int((TL + TH).sum())                    # slot-tiles per core
    L = T * P

    chunk_base = np.zeros(cpc + 1, np.int64)
    np.cumsum((TL + TH) * P, out=chunk_base[1:])
    base = np.zeros(nchg * 2, np.int64)         # flat base per (chunk, stream)
    base[0::2] = chunk_base[:-1][pos_of]
    base[1::2] = chunk_base[:-1][pos_of] + TL[pos_of] * P

    # rank of each edge within its (chunk, stream) group
    eorder = np.argsort(grp, kind="stable")
    grp_sorted = grp[eorder]
    starts = np.searchsorted(grp_sorted, np.arange(nchg * 2))
    rank_sorted = np.arange(len(rows)) - starts[grp_sorted]
    rank = np.empty(len(rows), np.int64)
    rank[eorder] = rank_sorted

    flat_pos = base[grp] + rank                 # position in the core's flat list
    core = core_of[g]

    flat_idx = np.zeros((ncores, L), np.int16)  # pads gather row 0 harmlessly
    flat_rowl = np.zeros((ncores, L), np.float32)
    flat_vals = np.zeros((ncores, L), np.float32)
    flat_idx[core, flat_pos] = (cols - hi * CSPLIT).astype(np.int16)
    flat_rowl[core, flat_pos] = q.astype(np.float32)
    flat_vals[core, flat_pos] = vals

    # device table layouts
    idx16 = np.tile(
        flat_idx.reshape(ncores, L // 16, 16).transpose(0, 2, 1), (1, 8, 1)
    )                                           # [ncores, 128, L/16]
    rowl_t = flat_rowl.reshape(ncores, T, P).transpose(0, 2, 1).copy()  # [nc,128,T]
    vals_t = flat_vals.reshape(ncores, T, P).transpose(0, 2, 1).copy()

    return {
        "nchg": nchg, "cpc": cpc, "n_pad": n_pad, "T": T,
        "TL": TL, "TH": TH, "ns": ns2.reshape(-1), "asg": asg,
        "idx16": idx16, "rowl": rowl_t, "vals": vals_t,
    }


NQ = 4  # SWDGE queues: each uses a distinct Q7 core pair for desc-gen
HOST_SEL_MOD = 2  # positions c with c % HOST_SEL_MOD == 0 load host-built sel


def _host_sel_layout(TL, TH, cpc):
    """Tile offsets of host-built sel chunks within the packed sel stream."""
    off = {}
    w = 0
    for c in range(cpc):
        if c % HOST_SEL_MOD == 0:
            off[c] = w
            w += int(TL[c] + TH[c])
    return off, w


def _build_program(TL, TH, cpc, n_pad, T, gdt, nreal_sched, selw):
    """Emit the SPMD Bass program (identical for all cores)."""
    nc = bacc.Bacc("TRN2", num_swdge_queues=NQ, dynamic_dma_scratch_size=65536)
    Tmax = int((TL + TH).max())
    S16 = T * 8
    sel_off, _ = _host_sel_layout(TL, TH, cpc)

    xg = nc.dram_tensor("xg", [n_pad, D], gdt, kind="ExternalInput")
    xeyet = nc.dram_tensor("xeyet", [P, cpc * 2 * P], gdt, kind="ExternalInput")
    idx16 = nc.dram_tensor("idx16", [P, S16], I16, kind="ExternalInput")
    rowl = nc.dram_tensor("rowl", [P, T], gdt, kind="ExternalInput")
    valst = nc.dram_tensor("valst", [P, T], gdt, kind="ExternalInput")
    iota = nc.dram_tensor("iota", [P, Tmax * P], gdt, kind="ExternalInput")
    seld = nc.dram_tensor("seld", [P, selw * P], gdt, kind="ExternalInput")
    ident = nc.dram_tensor("ident", [P, P], F32, kind="ExternalInput")
    wlin = nc.dram_tensor("wlin", [P, 2 * HALF], F32, kind="ExternalInput")
    weye = nc.dram_tensor("weye", [P, 2 * HALF], gdt, kind="ExternalInput")
    bias = nc.dram_tensor("bias", [P, DO], F32, kind="ExternalInput")
    out = nc.dram_tensor("out", [cpc * P, DO], gdt, kind="ExternalOutput")

    with tile.TileContext(nc) as tc:
        with (
            tc.tile_pool(name="const", bufs=1) as cp,
            tc.tile_pool(name="eye", bufs=3) as eyep,
            tc.tile_pool(name="sel", bufs=6) as selp,
            tc.tile_pool(name="work", bufs=3) as wp,
            tc.tile_pool(name="pagg", bufs=2, space="PSUM") as pagg,
            tc.tile_pool(name="ptr", bufs=2, space="PSUM") as ptr,
            tc.tile_pool(name="pout", bufs=2, space="PSUM") as pout,
        ):
            idx_sb = cp.tile([P, S16], I16)
            nc.sync.dma_start(idx_sb[:], idx16[:])
            rowl_sb = cp.tile([P, T], gdt)
            nc.sync.dma_start(rowl_sb[:], rowl[:])
            vals_sb = cp.tile([P, T], gdt)
            nc.sync.dma_start(vals_sb[:], valst[:])
            iota_sb = cp.tile([P, Tmax, P], gdt)
            nc.sync.dma_start(iota_sb[:].rearrange("p t o -> p (t o)"), iota[:])
            ident_sb = cp.tile([P, P], F32)
            nc.sync.dma_start(ident_sb[:], ident[:])
            wlin_sb = cp.tile([P, 2 * HALF], F32)
            nc.sync.dma_start(wlin_sb[:], wlin[:])
            weye_sb = cp.tile([P, 2 * HALF], gdt)
            nc.sync.dma_start(weye_sb[:], weye[:])
            bias_sb = cp.tile([P, DO], F32)
            nc.sync.dma_start(bias_sb[:], bias[:])

            # Fixed msg ring: tile-object reuse gives exact double-buffer
            # deps; the one-time memset keeps un-gathered trailing slots of
            # partial tiles finite (their sel columns are zero).
            NMSG = 10
            mtiles = []
            for i in range(NMSG):
                mt = cp.tile([P, Tmax, D], gdt, tag=f"msg{i}")
                nc.vector.memset(mt[:], 0.0)
                mtiles.append(mt)
            soff = 0
            qcount = 0
            for c in range(cpc):
                tl, th = int(TL[c]), int(TH[c])
                tcnt = tl + th
                msg = mtiles[c % NMSG]
                IMAX = 1024  # dma_gather crashes HW above 1024 indices/call
                for st_, base_t, nsc in (
                    (0, soff, int(nreal_sched[2 * c])),
                    (1, soff + tl, int(nreal_sched[2 * c + 1])),
                ):
                    src = xg[0:CSPLIT, :] if st_ == 0 else xg[CSPLIT:n_pad, :]
                    # balanced tile-aligned splits (split points must be
                    # multiples of P so each call's dst starts at partition 0)
                    ncalls = -(-nsc // IMAX)
                    tpc = -(-(-(-nsc // P)) // ncalls)  # tiles per call
                    for ci in range(ncalls):
                        d = ci * tpc * P
                        num = min(tpc * P, nsc - d)
                        t0 = base_t + d // P
                        nt = -(-num // P)
                        nc.gpsimd.dma_gather(
                            msg[:, t0 - soff : t0 - soff + nt, :], src,
                            idx_sb[:, 8 * base_t + d // 16 :
                                   8 * base_t + d // 16 + -(-num // 16)],
                            num, num, D,
                            queue_num=qcount % NQ,
                        )
                        qcount += 1
                ex = eyep.tile([P, 2, P], gdt, tag="eye")
                nc.sync.dma_start(
                    ex[:].rearrange("p h n -> p (h n)"),
                    xeyet[:, c * 2 * P : (c + 1) * 2 * P],
                )

                agg_ps = pagg.tile([P, D], F32, space="PSUM", tag="agg")
                sel_all = selp.tile([P, Tmax, P], gdt, tag="sel")
                if c in sel_off:
                    so = sel_off[c] * P
                    nc.sync.dma_start(
                        sel_all[:, 0:tcnt, :].rearrange("p t o -> p (t o)"),
                        seld[:, so : so + tcnt * P],
                    )
                else:
                    rb = (rowl_sb[:, soff : soff + tcnt]
                          .rearrange("p (t o) -> p t o", o=1)
                          .to_broadcast([P, tcnt, P]))
                    vb = (vals_sb[:, soff : soff + tcnt]
                          .rearrange("p (t o) -> p t o", o=1)
                          .to_broadcast([P, tcnt, P]))
                    nc.vector.tensor_tensor(
                        out=sel_all[:, 0:tcnt, :], in0=iota_sb[:, 0:tcnt, :],
                        in1=rb, op=mybir.AluOpType.is_equal,
                    )
                    nc.vector.tensor_tensor(
                        out=sel_all[:, 0:tcnt, :], in0=sel_all[:, 0:tcnt, :],
                        in1=vb, op=mybir.AluOpType.mult,
                    )
                for t in range(tcnt):
                    nc.tensor.matmul(
                        out=agg_ps[:], lhsT=sel_all[:, t, :], rhs=msg[:, t, :],
                        start=(t == 0), stop=(t == tcnt - 1),
                    )
                soff += tcnt

                agg_sb = wp.tile([P, D], F32, tag="aggsb")
                nc.scalar.copy(agg_sb[:], agg_ps[:])
                aggT = wp.tile([P, 2, P], F32, tag="aggT")
                for h in range(2):
                    tp = ptr.tile([P, P], F32, space="PSUM", tag="tp")
                    nc.tensor.transpose(
                        out=tp[:], in_=agg_sb[:, h * P : (h + 1) * P],
                        identity=ident_sb[:],
                    )
                    nc.scalar.copy(aggT[:, h, :], tp[:])

                # device column layout: h*128 + 64*is_eye + 32*(b%2) + o
                out_ps = pout.tile([P, DO], F32, space="PSUM", tag="outps")
                for h in range(2):
                    nc.tensor.matmul(
                        out=out_ps[:, h * P : h * P + 2 * HALF],
                        lhsT=aggT[:, h, :], rhs=wlin_sb[:],
                        start=True, stop=True,
                    )
                    nc.tensor.matmul(
                        out=out_ps[:, h * P + 2 * HALF : (h + 1) * P],
                        lhsT=ex[:, h, :], rhs=weye_sb[:],
                        start=True, stop=True,
                    )
                out_sb = wp.tile([P, DO], gdt, tag="outsb")
                nc.vector.tensor_add(out=out_sb[:], in0=out_ps[:], in1=bias_sb[:])
                nc.sync.dma_start(out[c * P : (c + 1) * P, :], out_sb[:])

    nc.compile()
    return nc


def _prepare_inputs(x, vals, W_lin, b_lin, W_eye, b_eye, rows, cols, n, ncores, gdt_np):
    sch = _schedule(rows, cols, vals, n, ncores)
    nchg, cpc, n_pad = sch["nchg"], sch["cpc"], sch["n_pad"]

    b_, n_, c_ = x.shape
    xg = np.zeros((n_pad, b_ * c_), dtype=np.float32)
    xg[:n_] = np.ascontiguousarray(x.transpose(1, 0, 2)).reshape(n_, b_ * c_)

    # bias in device column layout: h*128 + 64*is_eye + 32*(b%2) + o
    bias_row = np.zeros(DO, dtype=np.float32)
    for h in range(2):
        for bb in range(2):
            bias_row[h * 128 + bb * 32 : h * 128 + bb * 32 + 32] = b_lin
            bias_row[h * 128 + 64 + bb * 32 : h * 128 + 64 + bb * 32 + 32] = b_eye
    bias_full = np.tile(bias_row[None, :], (P, 1))

    tmax = int((sch["TL"] + sch["TH"]).max())
    iota = np.tile(np.arange(P, dtype=np.float32)[None, None, :], (P, tmax, 1)).reshape(
        P, tmax * P
    )
    ident = np.eye(P, dtype=np.float32)
    def blockdiag(w):
        wt = np.ascontiguousarray(w.T.astype(np.float32))   # [64, 32]
        bd = np.zeros((P, 2 * HALF), dtype=np.float32)
        bd[:CIN, :HALF] = wt
        bd[CIN:, HALF:] = wt
        return bd

    wlinT = blockdiag(W_lin)
    weyeT = blockdiag(W_eye)

    TL, TH, T = sch["TL"], sch["TH"], sch["T"]
    sel_off, selw = _host_sel_layout(TL, TH, cpc)
    host_tiles = np.zeros(T, dtype=bool)
    tb = 0
    for c in range(cpc):
        if c in sel_off:
            host_tiles[tb : tb + int(TL[c] + TH[c])] = True
        tb += int(TL[c] + TH[c])
    ht = np.nonzero(host_tiles)[0]
    dstt = np.cumsum(host_tiles) - 1            # dst tile index per src tile

    in_maps = []
    # [b, cc, chunk, q] view of x for the packed transposed eye input
    xt_all = np.ascontiguousarray(
        xg.reshape(nchg, P, b_, c_).transpose(2, 3, 0, 1)
    )
    for k in range(ncores):
        chunks = sch["asg"][:, k]
        # xeyet[bb*64+cc, (chunk, h, q)] = x[2h+bb, node(chunk, q), cc]
        xe = xt_all[:, :, chunks, :].reshape(2, 2, c_, cpc, P)
        xeyet = np.ascontiguousarray(
            xe.transpose(1, 2, 3, 0, 4).reshape(P, cpc * 2 * P)
        )
        # host-built sel tiles (final bf16 one-hot * val), packed per core
        rowl_k, vals_k = sch["rowl"][k], sch["vals"][k]
        selh = np.zeros((P, selw, P), dtype=gdt_np)
        pp = np.repeat(np.arange(P), len(ht))
        tt = np.tile(ht, P)
        selh[pp, dstt[tt], rowl_k[pp, tt].astype(np.int64)] = (
            vals_k[pp, tt].astype(gdt_np)
        )
        in_maps.append({
            "xg": xg.astype(gdt_np),
            "xeyet": xeyet.astype(gdt_np),
            "idx16": sch["idx16"][k],
            "rowl": rowl_k.astype(gdt_np),
            "valst": vals_k.astype(gdt_np),
            "iota": iota.astype(gdt_np),
            "seld": selh.reshape(P, selw * P),
            "ident": ident,
            "wlin": wlinT, "weye": weyeT.astype(gdt_np), "bias": bias_full,
        })
    return sch, in_maps, selw


def _assemble(results, sch, n, ncores):
    nchg, cpc = sch["nchg"], sch["cpc"]
    out_pad = np.zeros((nchg, P, DO), dtype=np.float32)
    for k in range(ncores):
        chunks = sch["asg"][:, k]
        out_pad[chunks] = results[k]["out"].astype(np.float32).reshape(cpc, P, DO)
    flat = out_pad.reshape(nchg * P, DO)[:n]          # [N, device-layout cols]
    # invert device column layout -> [b, oc]
    perm = np.empty(DO, dtype=np.int64)
    for b in range(B):
        h, bb = b // 2, b % 2
        oc = np.arange(2 * HALF)
        dev = np.where(
            oc < HALF,
            h * 128 + bb * 32 + oc,
            h * 128 + 64 + bb * 32 + (oc - HALF),
        )
        perm[b * 2 * HALF + oc] = dev
    flat = flat[:, perm]
    return np.ascontiguousarray(
        flat.reshape(n, B, 2 * HALF).transpose(1, 0, 2)
    )


def _run(inputs, trace=False, trace_kwargs=None):
    x = np.asarray(inputs["x"], dtype=np.float32)
    vals = np.asarray(inputs["vals"], dtype=np.float32)
    rows = np.asarray(inputs["rows"])
    cols = np.asarray(inputs["cols"])
    W_lin = np.asarray(inputs["W_lin"], dtype=np.float32)
    b_lin = np.asarray(inputs["b_lin"], dtype=np.float32)
    W_eye = np.asarray(inputs["W_eye"], dtype=np.float32)
    b_eye = np.asarray(inputs["b_eye"], dtype=np.float32)

    import ml_dtypes

    sch, in_maps, selw = _prepare_inputs(
        x, vals, W_lin, b_lin, W_eye, b_eye, rows, cols, N, NCORES,
        ml_dtypes.bfloat16,
    )
    nc = _build_program(
        sch["TL"], sch["TH"], sch["cpc"], sch["n_pad"], sch["T"], BF16,
        sch["ns"], selw,
    )
    res = run_bass_kernel_spmd(
        nc, in_maps, core_ids=list(range(NCORES)),
        trace=trace, **(trace_kwargs or {}),
    )
    out = _assemble(res.results, sch, N, NCORES)
    return out, res


def kernel(**inputs) -> np.ndarray:
    out, _ = _run(inputs, trace=False)
    return out

